# revision 5
# baseline (speedup 1.0000x reference)
"""MoE layer (E=8, top-2, SwiGLU experts) on 8 trn2 NeuronCores.

Strategy (expert parallel, host-routed):
  - Router (flat @ router_w.T, top-2, softmax) is computed on host in fp32;
    it is tiny (33 MFLOP) and must match the reference's expert selection
    exactly (min top2-vs-3rd logit gap on these inputs is ~1e-4, far above
    fp32 matmul noise ~1e-6).
  - Tokens are dispatched to core e = expert e (the "all-to-all"), padded to
    a fixed capacity CAP. Each core runs a dense bf16 SwiGLU FFN for its
    expert over its routed tokens: yT = w2T.T @ (silu(w1T.T@xT) * (w3T.T@xT)).
    All tensors are pre-transposed on host so the contraction dim lands on
    SBUF partitions with no on-device transposes.
  - Host combines: out[tok] += combine_weight * y (each token appears in
    exactly 2 experts' outputs).

Compute dtype bf16 (PE runs fp32 at 1/4 rate), fp32 PSUM accumulation,
fp32 output.
"""

import os
import numpy as np
import ml_dtypes

B, S, D, H, E = 2, 2048, 1024, 2048, 8
T = B * S
TOP_K = 2
P = 128
NTOK = 512  # token chunk (matmul free dim / one PSUM bank of fp32)

_cache = {}

# set by the last kernel() call when tracing is enabled (KERNEL_TRACE=1)
LAST_RESULTS = None


def _build_nc(cap, act="silu"):
    import concourse.mybir as mybir
    import concourse.tile as tile
    from concourse import bacc

    bf16 = mybir.dt.bfloat16
    f32 = mybir.dt.float32
    # "sigmoid" exists only for CoreSim smoke tests (sim lacks Silu)
    Silu = (
        mybir.ActivationFunctionType.Silu
        if act == "silu"
        else mybir.ActivationFunctionType.Sigmoid
    )

    D_T, H_T = D // P, H // P  # 8, 16 partition tiles
    chunks = [(s, min(NTOK, cap - s)) for s in range(0, cap, NTOK)]

    nc = bacc.Bacc()
    xT_d = nc.declare_dram_parameter("xT", [D, cap], bf16, isOutput=False)
    w1T_d = nc.declare_dram_parameter("w1T", [D, H], bf16, isOutput=False)
    w3T_d = nc.declare_dram_parameter("w3T", [D, H], bf16, isOutput=False)
    w2T_d = nc.declare_dram_parameter("w2T", [H, D], bf16, isOutput=False)
    yT_d = nc.declare_dram_parameter("yT", [D, cap], f32, isOutput=True)

    with tile.TileContext(nc) as tc:
        with (
            tc.tile_pool(name="wpool", bufs=1) as wpool,
            tc.tile_pool(name="xpool", bufs=2) as xpool,
            tc.tile_pool(name="hpool", bufs=2) as hpool,
            tc.tile_pool(name="gpool", bufs=4) as gpool,
            tc.tile_pool(name="opool", bufs=4) as opool,
            tc.tile_pool(name="pspool", bufs=2, space="PSUM") as pspool,
        ):
            # Resident weights: w1T/w3T as 8 slabs [128, H], w2T as 16 slabs
            # [128, D]; 12 MB bf16 total. Emit w1/w3 first (needed by stage 1).
            w1s, w3s, w2s = [], [], []
            for d in range(D_T):
                w1t = wpool.tile([P, H], bf16, tag=f"w1_{d}", name=f"w1_{d}")
                nc.sync.dma_start(w1t[:], w1T_d[d * P:(d + 1) * P, :])
                w1s.append(w1t)
            for d in range(D_T):
                w3t = wpool.tile([P, H], bf16, tag=f"w3_{d}", name=f"w3_{d}")
                nc.sync.dma_start(w3t[:], w3T_d[d * P:(d + 1) * P, :])
                w3s.append(w3t)
            for h in range(H_T):
                w2t = wpool.tile([P, D], bf16, tag=f"w2_{h}", name=f"w2_{h}")
                nc.sync.dma_start(w2t[:], w2T_d[h * P:(h + 1) * P, :])
                w2s.append(w2t)

            for (s0, n) in chunks:
                # activations for this token chunk: 8 tiles [128, n]
                xs = []
                for d in range(D_T):
                    xt = xpool.tile([P, NTOK], bf16, tag=f"x_{d}", name=f"x_{d}")
                    nc.sync.dma_start(xt[:, :n], xT_d[d * P:(d + 1) * P, s0:s0 + n])
                    xs.append(xt)

                # stage 1: hT[h] = silu(w1T.T@xT) * (w3T.T@xT)  -> [128, n] bf16
                hts = []
                for h in range(H_T):
                    pg = pspool.tile([P, NTOK], f32, tag="pg", name="pg")
                    for d in range(D_T):
                        nc.tensor.matmul(
                            pg[:, :n],
                            lhsT=w1s[d][:, h * P:(h + 1) * P],
                            rhs=xs[d][:, :n],
                            start=(d == 0),
                            stop=(d == D_T - 1),
                        )
                    pu = pspool.tile([P, NTOK], f32, tag="pu", name="pu")
                    for d in range(D_T):
                        nc.tensor.matmul(
                            pu[:, :n],
                            lhsT=w3s[d][:, h * P:(h + 1) * P],
                            rhs=xs[d][:, :n],
                            start=(d == 0),
                            stop=(d == D_T - 1),
                        )
                    g = gpool.tile([P, NTOK], bf16, tag="g", name="g")
                    nc.scalar.activation(g[:, :n], pg[:, :n], Silu)
                    ht = hpool.tile([P, NTOK], bf16, tag=f"h_{h}", name=f"h_{h}")
                    nc.vector.tensor_mul(out=ht[:, :n], in0=g[:, :n], in1=pu[:, :n])
                    hts.append(ht)

                # stage 2: yT[do] = sum_h w2T[h,do].T @ hT[h]  -> [128, n] f32
                for do in range(D_T):
                    py = pspool.tile([P, NTOK], f32, tag="py", name="py")
                    for h in range(H_T):
                        nc.tensor.matmul(
                            py[:, :n],
                            lhsT=w2s[h][:, do * P:(do + 1) * P],
                            rhs=hts[h][:, :n],
                            start=(h == 0),
                            stop=(h == H_T - 1),
                        )
                    ot = opool.tile([P, NTOK], f32, tag="o", name="o")
                    nc.vector.tensor_copy(ot[:, :n], py[:, :n])
                    nc.sync.dma_start(yT_d[do * P:(do + 1) * P, s0:s0 + n], ot[:, :n])

    nc.finalize()
    return nc


def kernel(x, router_w, w1, w2, w3):
    global LAST_RESULTS
    from concourse.bass_utils import run_bass_kernel_spmd

    x = np.ascontiguousarray(np.asarray(x, dtype=np.float32))
    router_w = np.asarray(router_w, dtype=np.float32)
    flat = x.reshape(T, D)

    # ---- host router (fp32, matches reference math) ----
    logits = flat @ router_w.T                      # [T, E]
    rows = np.arange(T)
    i1 = np.argmax(logits, axis=1)
    v1 = logits[rows, i1]
    masked = logits.copy()
    masked[rows, i1] = -np.inf
    i2 = np.argmax(masked, axis=1)
    v2 = masked[rows, i2]
    # softmax over the two selected logits (v1 >= v2)
    e2 = np.exp(v2 - v1)
    wt1 = 1.0 / (1.0 + e2)
    wt2 = e2 / (1.0 + e2)

    # ---- dispatch: token lists per expert ----
    idxs, wts = [], []
    for e in range(E):
        m1 = i1 == e
        m2 = i2 == e
        idx = np.nonzero(m1 | m2)[0]
        w = np.where(m1[idx], wt1[idx], wt2[idx]).astype(np.float32)
        idxs.append(idx)
        wts.append(w)
    max_cnt = max(len(i) for i in idxs)
    cap = max(NTOK, -(-max_cnt // P) * P)

    if cap not in _cache:
        _cache[cap] = _build_nc(cap)
    nc = _cache[cap]

    # ---- per-core inputs (bf16, pre-transposed) ----
    bf = ml_dtypes.bfloat16
    in_maps = []
    for e in range(E):
        idx = idxs[e]
        xTe = np.zeros((D, cap), dtype=bf)
        xTe[:, :len(idx)] = flat[idx].T.astype(bf)
        in_maps.append({
            "xT": xTe,
            "w1T": np.ascontiguousarray(w1[e].T).astype(bf),
            "w3T": np.ascontiguousarray(w3[e].T).astype(bf),
            "w2T": np.ascontiguousarray(w2[e].T).astype(bf),
        })

    trace = os.environ.get("KERNEL_TRACE", "0") == "1"
    kwargs = {}
    if trace:
        kwargs = dict(trace=True, trace_cores=list(range(E)))
    res = run_bass_kernel_spmd(nc, in_maps, core_ids=list(range(E)), **kwargs)
    LAST_RESULTS = res

    # ---- combine (the "all-to-all" return + weighted sum) ----
    out = np.zeros((T, D), dtype=np.float32)
    for e in range(E):
        idx = idxs[e]
        yT = res.results[e]["yT"]                   # [D, cap] f32
        out[idx] += wts[e][:, None] * yT[:, :len(idx)].T
    return out.reshape(B, S, D)


# revision 8
# speedup vs baseline: 1.0990x; 1.0990x over previous
"""MoE layer (E=8, top-2, SwiGLU experts) on 8 trn2 NeuronCores.

Strategy (expert parallel, host-routed):
  - Router (flat @ router_w.T, top-2, softmax) is computed on host in fp32;
    it is tiny (33 MFLOP) and must match the reference's expert selection
    exactly (min top2-vs-3rd logit gap on these inputs is ~1e-4, far above
    fp32 matmul noise ~1e-6).
  - Tokens are dispatched to core e = expert e (the "all-to-all"), padded to
    a fixed capacity CAP. Each core runs a dense bf16 SwiGLU FFN for its
    expert over its routed tokens: yT = w2T.T @ (silu(w1T.T@xT) * (w3T.T@xT)).
    All tensors are pre-transposed on host so the contraction dim lands on
    SBUF partitions with no on-device transposes.
  - Host combines: out[tok] += combine_weight * y (each token appears in
    exactly 2 experts' outputs).

Compute dtype bf16 (PE runs fp32 at 1/4 rate), fp32 PSUM accumulation,
fp32 output.
"""

import os
import numpy as np
import ml_dtypes

B, S, D, H, E = 2, 2048, 1024, 2048, 8
T = B * S
TOP_K = 2
P = 128
NTOK = 512  # token chunk (matmul free dim / one PSUM bank of fp32)

_cache = {}

# set by the last kernel() call when tracing is enabled (KERNEL_TRACE=1)
LAST_RESULTS = None


def _build_nc(cap, act="silu"):
    import concourse.mybir as mybir
    import concourse.tile as tile
    from concourse import bacc

    bf16 = mybir.dt.bfloat16
    f32 = mybir.dt.float32
    # "sigmoid" exists only for CoreSim smoke tests (sim lacks Silu)
    Silu = (
        mybir.ActivationFunctionType.Silu
        if act == "silu"
        else mybir.ActivationFunctionType.Sigmoid
    )

    D_T, H_T = D // P, H // P  # 8, 16 partition tiles
    HQ = 4  # w1/w3 streamed in column packs of HQ*P = 512 (fine-grained deps)

    # equal-ish token chunks (each a multiple of 128, at most NTOK) so the
    # matmul free dim stays balanced across chunks
    n_chunks = -(-cap // NTOK)
    base, rem = divmod(cap // P, n_chunks)
    sizes = [(base + (1 if i < rem else 0)) * P for i in range(n_chunks)]
    chunks, s = [], 0
    for n in sizes:
        chunks.append((s, n))
        s += n

    nc = bacc.Bacc()
    xT_d = nc.declare_dram_parameter("xT", [D, cap], bf16, isOutput=False)
    w1T_d = nc.declare_dram_parameter("w1T", [D, H], bf16, isOutput=False)
    w3T_d = nc.declare_dram_parameter("w3T", [D, H], bf16, isOutput=False)
    w2T_d = nc.declare_dram_parameter("w2T", [H, D], bf16, isOutput=False)
    yT_d = nc.declare_dram_parameter("yT", [D, cap], f32, isOutput=True)

    with tile.TileContext(nc) as tc:
        with (
            tc.tile_pool(name="wpool", bufs=1) as wpool,
            tc.tile_pool(name="xpool", bufs=2) as xpool,
            tc.tile_pool(name="hpool", bufs=2) as hpool,
            tc.tile_pool(name="gpool", bufs=4) as gpool,
            tc.tile_pool(name="opool", bufs=4) as opool,
            tc.tile_pool(name="pspool", bufs=2, space="PSUM") as pspool,
        ):
            # Resident weights. w1/w3 stream as [128, HQ*P] column packs in the
            # order stage 1 consumes them, so the PE can start after ~1.5 MB
            # instead of waiting for all 12 MB. First chunk's x loads first.
            xs0 = []
            for d in range(D_T):
                xt = xpool.tile([P, NTOK], bf16, tag=f"x_{d}", name=f"x_{d}")
                nc.sync.dma_start(xt[:, :chunks[0][1]],
                                  xT_d[d * P:(d + 1) * P, 0:chunks[0][1]])
                xs0.append(xt)
            w1q = [[None] * (H_T // HQ) for _ in range(D_T)]
            w3q = [[None] * (H_T // HQ) for _ in range(D_T)]
            for hq in range(H_T // HQ):
                c0, c1 = hq * HQ * P, (hq + 1) * HQ * P
                for d in range(D_T):
                    t = wpool.tile([P, HQ * P], bf16, tag=f"w1_{d}_{hq}", name=f"w1_{d}_{hq}")
                    nc.sync.dma_start(t[:], w1T_d[d * P:(d + 1) * P, c0:c1])
                    w1q[d][hq] = t
                for d in range(D_T):
                    t = wpool.tile([P, HQ * P], bf16, tag=f"w3_{d}_{hq}", name=f"w3_{d}_{hq}")
                    nc.sync.dma_start(t[:], w3T_d[d * P:(d + 1) * P, c0:c1])
                    w3q[d][hq] = t
            w2s = []
            for h in range(H_T):
                w2t = wpool.tile([P, D], bf16, tag=f"w2_{h}", name=f"w2_{h}")
                nc.sync.dma_start(w2t[:], w2T_d[h * P:(h + 1) * P, :])
                w2s.append(w2t)

            for ci, (s0, n) in enumerate(chunks):
                # activations for this token chunk: 8 tiles [128, n]
                if ci == 0:
                    xs = xs0
                else:
                    xs = []
                    for d in range(D_T):
                        xt = xpool.tile([P, NTOK], bf16, tag=f"x_{d}", name=f"x_{d}")
                        nc.sync.dma_start(xt[:, :n], xT_d[d * P:(d + 1) * P, s0:s0 + n])
                        xs.append(xt)

                # stage 1: hT[h] = silu(w1T.T@xT) * (w3T.T@xT)  -> [128, n] bf16
                hts = []
                for h in range(H_T):
                    hq, hr = divmod(h, HQ)
                    cs = slice(hr * P, (hr + 1) * P)
                    pg = pspool.tile([P, NTOK], f32, tag="pg", name="pg")
                    for d in range(D_T):
                        nc.tensor.matmul(
                            pg[:, :n],
                            lhsT=w1q[d][hq][:, cs],
                            rhs=xs[d][:, :n],
                            start=(d == 0),
                            stop=(d == D_T - 1),
                        )
                    pu = pspool.tile([P, NTOK], f32, tag="pu", name="pu")
                    for d in range(D_T):
                        nc.tensor.matmul(
                            pu[:, :n],
                            lhsT=w3q[d][hq][:, cs],
                            rhs=xs[d][:, :n],
                            start=(d == 0),
                            stop=(d == D_T - 1),
                        )
                    g = gpool.tile([P, NTOK], bf16, tag="g", name="g")
                    nc.scalar.activation(g[:, :n], pg[:, :n], Silu)
                    ht = hpool.tile([P, NTOK], bf16, tag=f"h_{h}", name=f"h_{h}")
                    nc.vector.tensor_mul(out=ht[:, :n], in0=g[:, :n], in1=pu[:, :n])
                    hts.append(ht)

                # stage 2: yT[do] = sum_h w2T[h,do].T @ hT[h]  -> [128, n] f32
                for do in range(D_T):
                    py = pspool.tile([P, NTOK], f32, tag="py", name="py")
                    for h in range(H_T):
                        nc.tensor.matmul(
                            py[:, :n],
                            lhsT=w2s[h][:, do * P:(do + 1) * P],
                            rhs=hts[h][:, :n],
                            start=(h == 0),
                            stop=(h == H_T - 1),
                        )
                    ot = opool.tile([P, NTOK], f32, tag="o", name="o")
                    nc.vector.tensor_copy(ot[:, :n], py[:, :n])
                    nc.sync.dma_start(yT_d[do * P:(do + 1) * P, s0:s0 + n], ot[:, :n])

    nc.finalize()
    return nc


def kernel(x, router_w, w1, w2, w3):
    global LAST_RESULTS
    from concourse.bass_utils import run_bass_kernel_spmd

    x = np.ascontiguousarray(np.asarray(x, dtype=np.float32))
    router_w = np.asarray(router_w, dtype=np.float32)
    flat = x.reshape(T, D)

    # ---- host router (fp32, matches reference math) ----
    logits = flat @ router_w.T                      # [T, E]
    rows = np.arange(T)
    i1 = np.argmax(logits, axis=1)
    v1 = logits[rows, i1]
    masked = logits.copy()
    masked[rows, i1] = -np.inf
    i2 = np.argmax(masked, axis=1)
    v2 = masked[rows, i2]
    # softmax over the two selected logits (v1 >= v2)
    e2 = np.exp(v2 - v1)
    wt1 = 1.0 / (1.0 + e2)
    wt2 = e2 / (1.0 + e2)

    # ---- dispatch: token lists per expert ----
    idxs, wts = [], []
    for e in range(E):
        m1 = i1 == e
        m2 = i2 == e
        idx = np.nonzero(m1 | m2)[0]
        w = np.where(m1[idx], wt1[idx], wt2[idx]).astype(np.float32)
        idxs.append(idx)
        wts.append(w)
    max_cnt = max(len(i) for i in idxs)
    cap = max(NTOK, -(-max_cnt // P) * P)

    if cap not in _cache:
        _cache[cap] = _build_nc(cap)
    nc = _cache[cap]

    # ---- per-core inputs (bf16, pre-transposed) ----
    bf = ml_dtypes.bfloat16
    in_maps = []
    for e in range(E):
        idx = idxs[e]
        xTe = np.zeros((D, cap), dtype=bf)
        xTe[:, :len(idx)] = flat[idx].T.astype(bf)
        in_maps.append({
            "xT": xTe,
            "w1T": np.ascontiguousarray(w1[e].T).astype(bf),
            "w3T": np.ascontiguousarray(w3[e].T).astype(bf),
            "w2T": np.ascontiguousarray(w2[e].T).astype(bf),
        })

    trace = os.environ.get("KERNEL_TRACE", "0") == "1"
    kwargs = {}
    if trace:
        kwargs = dict(trace=True, trace_cores=list(range(E)))
    res = run_bass_kernel_spmd(nc, in_maps, core_ids=list(range(E)), **kwargs)
    LAST_RESULTS = res

    # ---- combine (the "all-to-all" return + weighted sum) ----
    out = np.zeros((T, D), dtype=np.float32)
    for e in range(E):
        idx = idxs[e]
        yT = res.results[e]["yT"]                   # [D, cap] f32
        out[idx] += wts[e][:, None] * yT[:, :len(idx)].T
    return out.reshape(B, S, D)


# revision 9
# speedup vs baseline: 1.2218x; 1.1117x over previous
"""MoE layer (E=8, top-2, SwiGLU experts) on 8 trn2 NeuronCores.

Strategy (expert parallel, host-routed):
  - Router (flat @ router_w.T, top-2, softmax) is computed on host in fp32;
    it is tiny (33 MFLOP) and must match the reference's expert selection
    exactly (min top2-vs-3rd logit gap on these inputs is ~1e-4, far above
    fp32 matmul noise ~1e-6).
  - Tokens are dispatched to core e = expert e (the "all-to-all"), padded to
    a fixed capacity CAP. Each core runs a dense bf16 SwiGLU FFN for its
    expert over its routed tokens: yT = w2T.T @ (silu(w1T.T@xT) * (w3T.T@xT)).
    All tensors are pre-transposed on host so the contraction dim lands on
    SBUF partitions with no on-device transposes.
  - Host combines: out[tok] += combine_weight * y (each token appears in
    exactly 2 experts' outputs).

Compute dtype bf16 (PE runs fp32 at 1/4 rate), fp32 PSUM accumulation,
fp32 output.
"""

import os
import numpy as np
import ml_dtypes

B, S, D, H, E = 2, 2048, 1024, 2048, 8
T = B * S
TOP_K = 2
P = 128
NTOK = 512  # max token chunk (matmul free dim / one PSUM bank of fp32)

_cache = {}

# set by the last kernel() call when tracing is enabled (KERNEL_TRACE=1)
LAST_RESULTS = None


def _build_nc(cap, act="silu"):
    import concourse.mybir as mybir
    import concourse.tile as tile
    from concourse import bacc

    bf16 = mybir.dt.bfloat16
    f32 = mybir.dt.float32
    # "sigmoid" exists only for CoreSim smoke tests (sim lacks Silu)
    Silu = (
        mybir.ActivationFunctionType.Silu
        if act == "silu"
        else mybir.ActivationFunctionType.Sigmoid
    )

    D_T, H_T = D // P, H // P  # 8, 16 partition tiles
    HQ = 4       # w1/w3 stream in column packs of HQ*P = 512 columns
    W2Q = 8      # w2 streams in two packs of 8 h-slabs

    # equal-ish token chunks (multiples of 8, at most NTOK) so the matmul
    # free dim stays balanced across chunks
    n_chunks = -(-cap // NTOK)
    base, rem = divmod(cap // 8, n_chunks)
    sizes = [(base + (1 if i < rem else 0)) * 8 for i in range(n_chunks)]
    chunks, s = [], 0
    for n in sizes:
        chunks.append((s, n))
        s += n

    nc = bacc.Bacc()
    xT_d = nc.declare_dram_parameter("xT", [D, cap], bf16, isOutput=False)
    w1T_d = nc.declare_dram_parameter("w1T", [D, H], bf16, isOutput=False)
    w3T_d = nc.declare_dram_parameter("w3T", [D, H], bf16, isOutput=False)
    w2T_d = nc.declare_dram_parameter("w2T", [H, D], bf16, isOutput=False)
    yT_d = nc.declare_dram_parameter("yT", [D, cap], f32, isOutput=True)

    # partition-major views for single-DMA loads of multi-slab packs
    xT_p = xT_d[:].rearrange("(dt p) n -> p dt n", p=P)      # [128, 8, cap]
    w1T_p = w1T_d[:].rearrange("(dt p) h -> p dt h", p=P)    # [128, 8, H]
    w3T_p = w3T_d[:].rearrange("(dt p) h -> p dt h", p=P)
    w2T_p = w2T_d[:].rearrange("(ht p) c -> p ht c", p=P)    # [128, 16, D]

    with tile.TileContext(nc) as tc:
        with (
            tc.tile_pool(name="wpool", bufs=1) as wpool,
            tc.tile_pool(name="xpool", bufs=2) as xpool,
            tc.tile_pool(name="hpool", bufs=2) as hpool,
            tc.tile_pool(name="gpool", bufs=4) as gpool,
            tc.tile_pool(name="opool", bufs=4) as opool,
            tc.tile_pool(name="pspool", bufs=2, space="PSUM") as pspool,
        ):
            # Chunk-0 activations first (first gate group needs them), then
            # w1/w3 column packs interleaved in consumption order, then w2.
            # Each pack is ONE DMA so it spreads across all 16 SDMA engines
            # and keeps the HWDGE FIFO short.
            def load_x(n0, s0):
                xt = xpool.tile([P, D_T, NTOK], bf16, tag="x", name="x")
                nc.sync.dma_start(xt[:, :, :n0], xT_p[:, :, s0:s0 + n0])
                return xt

            xs = load_x(chunks[0][1], 0)
            w1q, w3q = [], []
            for hq in range(H_T // HQ):
                c0, c1 = hq * HQ * P, (hq + 1) * HQ * P
                t1 = wpool.tile([P, D_T, HQ * P], bf16, tag=f"w1_{hq}", name=f"w1_{hq}")
                nc.sync.dma_start(t1[:], w1T_p[:, :, c0:c1])
                w1q.append(t1)
                t3 = wpool.tile([P, D_T, HQ * P], bf16, tag=f"w3_{hq}", name=f"w3_{hq}")
                nc.sync.dma_start(t3[:], w3T_p[:, :, c0:c1])
                w3q.append(t3)
            w2q = []
            for wq in range(H_T // W2Q):
                t2 = wpool.tile([P, W2Q, D], bf16, tag=f"w2_{wq}", name=f"w2_{wq}")
                nc.sync.dma_start(t2[:], w2T_p[:, wq * W2Q:(wq + 1) * W2Q, :])
                w2q.append(t2)

            for ci, (s0, n) in enumerate(chunks):
                if ci > 0:
                    xs = load_x(n, s0)

                # stage 1: hT[h] = silu(w1T.T@xT) * (w3T.T@xT)  -> [128, n] bf16
                hts = []
                for h in range(H_T):
                    hq, hr = divmod(h, HQ)
                    cs = slice(hr * P, (hr + 1) * P)
                    pg = pspool.tile([P, NTOK], f32, tag="pg", name="pg")
                    for d in range(D_T):
                        nc.tensor.matmul(
                            pg[:, :n],
                            lhsT=w1q[hq][:, d, cs],
                            rhs=xs[:, d, :n],
                            start=(d == 0),
                            stop=(d == D_T - 1),
                        )
                    pu = pspool.tile([P, NTOK], f32, tag="pu", name="pu")
                    for d in range(D_T):
                        nc.tensor.matmul(
                            pu[:, :n],
                            lhsT=w3q[hq][:, d, cs],
                            rhs=xs[:, d, :n],
                            start=(d == 0),
                            stop=(d == D_T - 1),
                        )
                    g = gpool.tile([P, NTOK], bf16, tag="g", name="g")
                    nc.scalar.activation(g[:, :n], pg[:, :n], Silu)
                    ht = hpool.tile([P, NTOK], bf16, tag=f"h_{h}", name=f"h_{h}")
                    nc.vector.tensor_mul(out=ht[:, :n], in0=g[:, :n], in1=pu[:, :n])
                    hts.append(ht)

                # stage 2: yT[do] = sum_h w2T[h,do].T @ hT[h]  -> [128, n] f32
                for do in range(D_T):
                    py = pspool.tile([P, NTOK], f32, tag="py", name="py", bufs=3)
                    for h in range(H_T):
                        nc.tensor.matmul(
                            py[:, :n],
                            lhsT=w2q[h // W2Q][:, h % W2Q, do * P:(do + 1) * P],
                            rhs=hts[h][:, :n],
                            start=(h == 0),
                            stop=(h == H_T - 1),
                        )
                    ot = opool.tile([P, NTOK], f32, tag="o", name="o")
                    nc.vector.tensor_copy(ot[:, :n], py[:, :n])
                    # output DMAs ride the ACT HWDGE ring, away from the input loads
                    nc.scalar.dma_start(yT_d[do * P:(do + 1) * P, s0:s0 + n], ot[:, :n])

    nc.finalize()
    return nc


def kernel(x, router_w, w1, w2, w3):
    global LAST_RESULTS
    from concourse.bass_utils import run_bass_kernel_spmd

    x = np.ascontiguousarray(np.asarray(x, dtype=np.float32))
    router_w = np.asarray(router_w, dtype=np.float32)
    flat = x.reshape(T, D)

    # ---- host router (fp32, matches reference math) ----
    logits = flat @ router_w.T                      # [T, E]
    rows = np.arange(T)
    i1 = np.argmax(logits, axis=1)
    v1 = logits[rows, i1]
    masked = logits.copy()
    masked[rows, i1] = -np.inf
    i2 = np.argmax(masked, axis=1)
    v2 = masked[rows, i2]
    # softmax over the two selected logits (v1 >= v2)
    e2 = np.exp(v2 - v1)
    wt1 = 1.0 / (1.0 + e2)
    wt2 = e2 / (1.0 + e2)

    # ---- dispatch: token lists per expert ----
    idxs, wts = [], []
    for e in range(E):
        m1 = i1 == e
        m2 = i2 == e
        idx = np.nonzero(m1 | m2)[0]
        w = np.where(m1[idx], wt1[idx], wt2[idx]).astype(np.float32)
        idxs.append(idx)
        wts.append(w)
    max_cnt = max(len(i) for i in idxs)
    cap = max(NTOK, -(-max_cnt // 8) * 8)

    if cap not in _cache:
        _cache[cap] = _build_nc(cap)
    nc = _cache[cap]

    # ---- per-core inputs (bf16, pre-transposed) ----
    bf = ml_dtypes.bfloat16
    in_maps = []
    for e in range(E):
        idx = idxs[e]
        xTe = np.zeros((D, cap), dtype=bf)
        xTe[:, :len(idx)] = flat[idx].T.astype(bf)
        in_maps.append({
            "xT": xTe,
            "w1T": np.ascontiguousarray(w1[e].T).astype(bf),
            "w3T": np.ascontiguousarray(w3[e].T).astype(bf),
            "w2T": np.ascontiguousarray(w2[e].T).astype(bf),
        })

    trace = os.environ.get("KERNEL_TRACE", "0") == "1"
    kwargs = {}
    if trace:
        kwargs = dict(trace=True, trace_cores=list(range(E)))
    res = run_bass_kernel_spmd(nc, in_maps, core_ids=list(range(E)), **kwargs)
    LAST_RESULTS = res

    # ---- combine (the "all-to-all" return + weighted sum) ----
    out = np.zeros((T, D), dtype=np.float32)
    for e in range(E):
        idx = idxs[e]
        yT = res.results[e]["yT"]                   # [D, cap] f32
        out[idx] += wts[e][:, None] * yT[:, :len(idx)].T
    return out.reshape(B, S, D)
